# revision 7
# baseline (speedup 1.0000x reference)
"""Trainium2 Bass kernel for DualGraphConvolution.

reference math (N=8192, D=512):
    node_att = softmax(x @ node_w, axis=0)            # [N, 1]
    h        = x @ edge_w                             # [N, D]
    e        = h @ h.T ; masked where adj <= 0        # [N, N]
    edge_att = softmax(e, axis=1)                     # [N, N]
    out      = (adj * node_att * edge_att) @ (x @ weight) + bias

Distribution: row-shard the N dimension over 8 NeuronCores (1024 rows each).
Each core receives a *rotated* copy of the inputs (its own rows first) so the
SPMD program only ever uses static indices; column order of the j-contraction
is irrelevant because it is summed over.

Per core the kernel computes, for its 1024 rows r:
    m[r, j] = e[r, j] + (adj[r, j] - 1) * 1000        # masked-out cols pushed
                                                      # ~-1000 below the row max
    rowmax, t = exp(m - rowmax), Z = sum t            # online softmax over 4
                                                      # column sweeps of 2048
    O[r, :] = t @ support                             # support = x @ weight
    out = O * (exp(p_r) / (sum_k exp(p_k)) / Z) + bias  # p = x @ node_w

Matmul operands are fp16 (PE runs them at full rate; ~1e-3 relative error),
all accumulation/softmax state is fp32.
"""

import numpy as np

import concourse.bass as bass
import concourse.mybir as mybir
import concourse.tile as tile
from concourse import bacc
from concourse.bass_utils import run_bass_kernel_spmd

F16 = mybir.dt.float16
F32 = mybir.dt.float32
ALU = mybir.AluOpType
ACTF = mybir.ActivationFunctionType
AX = mybir.AxisListType

N = 8192
D = 512
NCORES = 8
JSWEEP = 2048  # columns per online-softmax sweep
NEG_INIT = -3.0e38
MASK_SHIFT = 1000.0  # adj==0 columns get e - 1000: far below row max, exp -> 0


def build_program(n=N, d=D, ncores=NCORES, jsweep=JSWEEP,
                  dbg_skip_main=False, dbg_skip_nodestats=False,
                  dbg_main_upto=None):
    loc = n // ncores          # rows owned by this core
    rb = loc // 128            # 128-row blocks per core
    kc = d // 128              # contraction chunks of 128
    nsweep = n // jsweep       # online-softmax sweeps
    jt = jsweep // 512         # 512-col j tiles per sweep
    jc = jsweep // 128         # 128-col j chunks per sweep
    rchunks = n // 128
    assert jc % 4 == 0

    nc = bacc.Bacc("TRN2", target_bir_lowering=False, debug=False,
                   num_devices=ncores)

    xt_d = nc.dram_tensor("xt", [kc, 128, n], F16, kind="ExternalInput")
    adj_d = nc.dram_tensor("adj", [loc, n], F16, kind="ExternalInput")
    ew_d = nc.dram_tensor("ew", [kc, 128, d], F16, kind="ExternalInput")
    wt_d = nc.dram_tensor("wt", [kc, 128, d], F16, kind="ExternalInput")
    nw_d = nc.dram_tensor("nw", [kc, 128, 1], F16, kind="ExternalInput")
    bias_d = nc.dram_tensor("biasb", [128, d], F32, kind="ExternalInput")
    id_d = nc.dram_tensor("ident", [128, 128], F16, kind="ExternalInput")
    out_d = nc.dram_tensor("out", [loc, d], F32, kind="ExternalOutput")

    with tile.TileContext(nc) as tc:
        with (
            tc.tile_pool(name="const", bufs=1) as constp,
            tc.tile_pool(name="big", bufs=1) as bigp,
            tc.tile_pool(name="vec", bufs=6) as vecp,
        ):
            ew_sb = constp.tile([128, kc, d], F16)
            nc.sync.dma_start(ew_sb[:], ew_d.rearrange("c p f -> p c f"))
            wt_sb = constp.tile([128, kc, d], F16)
            nc.sync.dma_start(wt_sb[:], wt_d.rearrange("c p f -> p c f"))
            nw_sb = constp.tile([128, kc, 1], F16)
            nc.sync.dma_start(nw_sb[:], nw_d.rearrange("c p f -> p c f"))
            bias_sb = constp.tile([128, d], F32)
            nc.sync.dma_start(bias_sb[:], bias_d[:])
            id_sb = constp.tile([128, 128], F16)
            nc.sync.dma_start(id_sb[:], id_d[:])

            hT_sb = bigp.tile([128, kc, n], F16)   # h[r, dd] at [dd%128, dd//128, r]
            sup_sb = bigp.tile([128, rchunks, d], F16)  # support[rc*128+p, f]
            p_sb = bigp.tile([128, rchunks], F32)  # p[rc*128+p] = x @ node_w

            # ---- phase 0: hT = edge_w.T-chunks @ xT, support = x @ weight ----
            xt_view = xt_d.rearrange("c p r -> p c r")
            with (
                tc.tile_pool(name="ph0", bufs=3) as ph0p,
                tc.tile_pool(name="ph0ps", bufs=2, space="PSUM") as ph0ps,
                tc.tile_pool(name="ph0ps1", bufs=2, space="PSUM") as ph0ps1,
            ):
                for rt in range(n // 512):
                    xt_t = ph0p.tile([128, kc, 512], F16, tag="xt")
                    nc.sync.dma_start(xt_t[:], xt_view[:, :, rt * 512:(rt + 1) * 512])
                    for dc in range(kc):
                        hps = ph0ps.tile([128, 512], F32, tag="hps")
                        for c in range(kc):
                            nc.tensor.matmul(
                                hps[:], ew_sb[:, c, dc * 128:(dc + 1) * 128],
                                xt_t[:, c, :], start=(c == 0), stop=(c == kc - 1))
                        nc.any.tensor_copy(
                            out=hT_sb[:, dc, rt * 512:(rt + 1) * 512], in_=hps[:])
                    for rs in range(4):
                        rch = rt * 4 + rs
                        sps = ph0ps.tile([128, d], F32, tag="sps")
                        for c in range(kc):
                            nc.tensor.matmul(
                                sps[:], xt_t[:, c, rs * 128:(rs + 1) * 128],
                                wt_sb[:, c, :], start=(c == 0), stop=(c == kc - 1))
                        nc.any.tensor_copy(out=sup_sb[:, rch, :], in_=sps[:])
                        pps = ph0ps1.tile([128, 1], F32, tag="pps")
                        for c in range(kc):
                            nc.tensor.matmul(
                                pps[:], xt_t[:, c, rs * 128:(rs + 1) * 128],
                                nw_sb[:, c, :], start=(c == 0), stop=(c == kc - 1))
                        nc.any.tensor_copy(out=p_sb[:, rch:rch + 1], in_=pps[:])

            # ---- node attention: scale0 = exp(p_loc) / sum(exp(p)) ----
            scale0 = bigp.tile([128, rb], F32)
            if dbg_skip_nodestats:
                nc.vector.memset(scale0[:], 1.0)
            else:
                pexp = bigp.tile([128, rchunks], F32)
                prow = vecp.tile([128, 1], F32, tag="prow")
                nc.scalar.activation(pexp[:], p_sb[:], ACTF.Exp,
                                     accum_out=prow[:])
                ones_col = constp.tile([128, 1], F32)
                nc.vector.memset(ones_col[:], 1.0)
                ones_row = constp.tile([1, 128], F32)
                nc.vector.memset(ones_row[:], 1.0)
                with tc.tile_pool(name="nps", bufs=1, space="PSUM") as npsp:
                    # cross-partition sum of prow via ones matmul
                    pz_ps = npsp.tile([1, 1], F32, tag="pzps")
                    nc.tensor.matmul(pz_ps[:], ones_col[:], prow[:])
                    pz = vecp.tile([1, 1], F32, tag="pz")
                    nc.any.tensor_copy(out=pz[:], in_=pz_ps[:])
                    pzi = vecp.tile([1, 1], F32, tag="pzi")
                    nc.vector.reciprocal(pzi[:], pz[:])
                    # broadcast [1,1] scalar to all partitions via K=1 matmul
                    pzb_ps = npsp.tile([128, 1], F32, tag="pzbps")
                    nc.tensor.matmul(pzb_ps[:], ones_row[:], pzi[:])
                    pzb = vecp.tile([128, 1], F32, tag="pzb")
                    nc.any.tensor_copy(out=pzb[:], in_=pzb_ps[:])
                nc.vector.tensor_scalar_mul(scale0[:], pexp[:, 0:rb], pzb[:])

            if dbg_skip_main:
                with tc.tile_pool(name="dbgo", bufs=2) as dbgo:
                    for b in range(rb):
                        o_t = dbgo.tile([128, d], F32, tag="o")
                        nc.vector.tensor_scalar_mul(o_t[:], sup_sb[:, b, :],
                                                    scale0[:, b:b + 1])
                        nc.sync.dma_start(out_d[b * 128:(b + 1) * 128, :],
                                          o_t[:])
                nc.finalize()
                return nc

            # ---- main loop: masked row softmax + SpMM, online over sweeps ----
            with (
                tc.tile_pool(name="adjp", bufs=2) as adjp,
                tc.tile_pool(name="mp", bufs=2) as mp,
                tc.tile_pool(name="tp", bufs=2) as tp,
                tc.tile_pool(name="ttp", bufs=3) as ttp,
                tc.tile_pool(name="accp", bufs=2) as accp,
                tc.tile_pool(name="outp", bufs=2) as outp,
                tc.tile_pool(name="epsp", bufs=3, space="PSUM") as epsp,
                tc.tile_pool(name="spsp", bufs=2, space="PSUM") as spsp,
                tc.tile_pool(name="ttpsp", bufs=2, space="PSUM") as ttpsp,
            ):
                for b in range(rb):
                    oacc = accp.tile([128, d], F32, tag="oacc")
                    zacc = vecp.tile([128, 1], F32, tag="zacc")
                    rmrun = None
                    for q in range(nsweep):
                        adj_t = adjp.tile([128, jsweep], F16, tag="adj")
                        nc.sync.dma_start(
                            adj_t[:],
                            adj_d[b * 128:(b + 1) * 128,
                                  q * jsweep:(q + 1) * jsweep])
                        m_t = mp.tile([128, jsweep], F32, tag="m")
                        mx_t = vecp.tile([128, jt], F32, tag="mx")
                        for j in range(jt):
                            eps = epsp.tile([128, 512], F32, tag="eps")
                            joff = q * jsweep + j * 512
                            for c in range(kc):
                                nc.tensor.matmul(
                                    eps[:], hT_sb[:, c, b * 128:(b + 1) * 128],
                                    hT_sb[:, c, joff:joff + 512],
                                    start=(c == 0), stop=(c == kc - 1))
                            # m = 1000*adj + e: kept cols sit ~1000 above
                            # masked ones, so exp(m - rowmax) masks exactly
                            nc.vector.scalar_tensor_tensor(
                                out=m_t[:, j * 512:(j + 1) * 512],
                                in0=adj_t[:, j * 512:(j + 1) * 512],
                                scalar=MASK_SHIFT, in1=eps[:],
                                op0=ALU.mult, op1=ALU.add)
                            nc.vector.reduce_max(
                                mx_t[:, j:j + 1],
                                m_t[:, j * 512:(j + 1) * 512], axis=AX.X)
                        rmq = vecp.tile([128, 1], F32, tag="rmq")
                        nc.vector.reduce_max(rmq[:], mx_t[:], axis=AX.X)
                        nrmq = vecp.tile([128, 1], F32, tag="nrmq")
                        nc.vector.tensor_scalar_mul(nrmq[:], rmq[:], -1.0)
                        t_t = tp.tile([128, jsweep], F16, tag="t")
                        zqp = vecp.tile([128, jt], F32, tag="zqp")
                        for j in range(jt):
                            nc.scalar.activation(
                                t_t[:, j * 512:(j + 1) * 512],
                                m_t[:, j * 512:(j + 1) * 512],
                                ACTF.Exp, bias=nrmq[:],
                                accum_out=zqp[:, j:j + 1])
                        zq = vecp.tile([128, 1], F32, tag="zq")
                        nc.vector.reduce_sum(zq[:], zqp[:], axis=AX.X)
                        # transpose t 128-chunks, SpMM against support
                        S = spsp.tile([128, d], F32, tag="S")
                        for g in range(jc // 4):
                            ttps = ttpsp.tile([128, 512], F16, tag="ttps")
                            for u in range(4):
                                ch = g * 4 + u
                                nc.tensor.transpose(
                                    ttps[:, u * 128:(u + 1) * 128],
                                    t_t[:, ch * 128:(ch + 1) * 128], id_sb[:])
                            tt_sb = ttp.tile([128, 512], F16, tag="tt")
                            if g % 2 == 0:
                                nc.scalar.activation(tt_sb[:], ttps[:], ACTF.Copy)
                            else:
                                nc.vector.tensor_copy(out=tt_sb[:], in_=ttps[:])
                            for u in range(4):
                                jchunk = q * jc + g * 4 + u
                                nc.tensor.matmul(
                                    S[:], tt_sb[:, u * 128:(u + 1) * 128],
                                    sup_sb[:, jchunk, :],
                                    start=(g == 0 and u == 0),
                                    stop=(g == jc // 4 - 1 and u == 3))
                        if q == 0:
                            nc.any.tensor_copy(out=oacc[:], in_=S[:])
                            nc.any.tensor_copy(out=zacc[:], in_=zq[:])
                            rmrun = rmq
                        else:
                            rmnew = vecp.tile([128, 1], F32, tag="rmnew")
                            nc.vector.tensor_tensor(rmnew[:], rmrun[:], rmq[:],
                                                    ALU.max)
                            dold = vecp.tile([128, 1], F32, tag="dold")
                            nc.vector.tensor_tensor(dold[:], rmrun[:], rmnew[:],
                                                    ALU.subtract)
                            dq = vecp.tile([128, 1], F32, tag="dq")
                            nc.vector.tensor_tensor(dq[:], rmq[:], rmnew[:],
                                                    ALU.subtract)
                            cold = vecp.tile([128, 1], F32, tag="cold")
                            nc.scalar.activation(cold[:], dold[:], ACTF.Exp)
                            cq = vecp.tile([128, 1], F32, tag="cq")
                            nc.scalar.activation(cq[:], dq[:], ACTF.Exp)
                            nc.vector.tensor_scalar_mul(oacc[:], oacc[:], cold[:])
                            nc.vector.scalar_tensor_tensor(
                                out=oacc[:], in0=S[:], scalar=cq[:],
                                in1=oacc[:], op0=ALU.mult, op1=ALU.add)
                            nc.vector.tensor_scalar_mul(zacc[:], zacc[:], cold[:])
                            nc.vector.scalar_tensor_tensor(
                                out=zacc[:], in0=zq[:], scalar=cq[:],
                                in1=zacc[:], op0=ALU.mult, op1=ALU.add)
                            rmrun = rmnew
                    zi = vecp.tile([128, 1], F32, tag="zi")
                    nc.vector.reciprocal(zi[:], zacc[:])
                    scb = vecp.tile([128, 1], F32, tag="scb")
                    nc.vector.tensor_tensor(scb[:], zi[:], scale0[:, b:b + 1],
                                            ALU.mult)
                    o_t = outp.tile([128, d], F32, tag="o")
                    nc.vector.scalar_tensor_tensor(
                        out=o_t[:], in0=oacc[:], scalar=scb[:],
                        in1=bias_sb[:], op0=ALU.mult, op1=ALU.add)
                    nc.sync.dma_start(out_d[b * 128:(b + 1) * 128, :], o_t[:])
    nc.finalize()
    return nc


def make_in_maps(x, adj, weight, bias, node_w, edge_w, n=N, d=D, ncores=NCORES):
    loc = n // ncores
    kc = d // 128
    xt = np.ascontiguousarray(x.T.astype(np.float16)).reshape(kc, 128, n)
    ew = np.ascontiguousarray(edge_w.astype(np.float16)).reshape(kc, 128, d)
    wt = np.ascontiguousarray(weight.astype(np.float16)).reshape(kc, 128, d)
    nw = np.ascontiguousarray(node_w.astype(np.float16)).reshape(kc, 128, 1)
    biasb = np.ascontiguousarray(
        np.broadcast_to(bias.astype(np.float32)[None, :], (128, d)))
    ident = np.eye(128, dtype=np.float16)
    adj16 = adj.astype(np.float16)
    in_maps = []
    for c in range(ncores):
        sh = c * loc
        xt_c = np.ascontiguousarray(np.roll(xt, -sh, axis=2))
        adj_c = np.ascontiguousarray(np.roll(adj16[sh:sh + loc], -sh, axis=1))
        in_maps.append({"xt": xt_c, "adj": adj_c, "ew": ew, "wt": wt, "nw": nw,
                        "biasb": biasb, "ident": ident})
    return in_maps


_CACHE = {}


def kernel(x, adj, weight, bias, node_w, edge_w):
    assert x.shape == (N, D) and adj.shape == (N, N)
    if "nc" not in _CACHE:
        _CACHE["nc"] = build_program()
    nc = _CACHE["nc"]
    in_maps = make_in_maps(x, adj, weight, bias, node_w, edge_w)
    res = run_bass_kernel_spmd(nc, in_maps, list(range(NCORES)))
    out = np.concatenate([res.results[c]["out"] for c in range(NCORES)], axis=0)
    return np.ascontiguousarray(out.astype(np.float32))
